# revision 41
# baseline (speedup 1.0000x reference)
"""AttentionHead kernel for TRN2, data-parallel over batch across 8 NeuronCores.

Per core: one batch element.  Host passes xT (=x.T, bf16, window-major,
partition-major rows so each window is one 6KB-per-partition DMA).
  qkT[128, t] = [Wq*scale | Wk].T @ xT   (fused q+k projection; rows 0-63 = q,
                rows 64-127 = k); q evac on ScalarE, k evac on DVE (parallel).
  v'[k, 65]   = per k-chunk: xT-chunk.T @ Wv (direct transposed projection,
                no PE transpose pass); col 64 = ones (softmax denominator).
  sT[k, q]    = k @ q.T  (no bias in f32)
  attnT       = exp(sT) * E   where E[p, j] = exp(bias_table[j-2048-p]) for
                0 <= j-2048-p < 2048 else 0.  The multiply runs on DVE in
                bf16 (2x mode); E==0 doubles as the exact causal mask.
  out'[65, q] = v'.T @ attnT  (incremental: each pair's two 512-col streams
                accumulate as soon as that pair's exp*E is done, so the PE
                pipelines with the Scalar engine's exp stream)
                -> host divides by row 64 (denominator) and transposes.
"""

import numpy as np
import ml_dtypes
from contextlib import ExitStack

import concourse.bass as bass
import concourse.bacc as bacc
from concourse import mybir
from concourse.tile import TileContext

B, T, C, H = 8, 2048, 768, 64
NCORES = 8
# bias strip: only columns [1536, 4096) of the natural [128, 4096] Toeplitz
# strip are ever addressed (j0 = 2048 + q0 - kc*128 >= 1664 always), so the
# strip is stored pre-shifted: col j' <-> rel distance d = j' - 512 - p.
RTW = 2560
BF = mybir.dt.bfloat16
F32 = mybir.dt.float32
AF = mybir.ActivationFunctionType

NT = T // 128   # 16 t-chunks
NCC = C // 128  # 6 c-chunks
NW = T // 512   # 4 q-windows
NWARM = 10


def build_nc():
    nc = bacc.Bacc()
    # x.T, window-major, partition-major: [NW, 128, NCC*512]
    xt = nc.declare_dram_parameter("xt", [NW, 128, NCC * 512], BF, isOutput=False)
    wqk = nc.declare_dram_parameter("wqk", [C, 2 * H], BF, isOutput=False)
    wv = nc.declare_dram_parameter("wv", [C, H], BF, isOutput=False)
    rt = nc.declare_dram_parameter("rt", [128, RTW], BF, isOutput=False)
    out = nc.declare_dram_parameter("out", [H + 1, T], F32, isOutput=True)

    with TileContext(nc) as tc, ExitStack() as ctx:
        _body(tc, ctx, xt, wqk, wv, rt, out)
    nc.compile()
    return nc


def _rt_pair_ap(rt_sb, j0, width):
    """AP over the bias strip shaped [128, 2, width]: bank b -> columns
    j0 - 128*b + f  (matching k-chunk pairs kc, kc+1)."""
    base = rt_sb[:, j0:j0 + width]
    return bass.AP(tensor=base.tensor, offset=base.offset,
                   ap=[base.ap[0], [-128, 2], base.ap[1]])


def _body(tc, ctx, xt, wqk, wv, rt, out):
    nc = tc.nc
    const = ctx.enter_context(tc.tile_pool(name="const", bufs=1))
    big = ctx.enter_context(tc.tile_pool(name="big", bufs=1))
    work = ctx.enter_context(tc.tile_pool(name="work", bufs=18))
    psum_qk = ctx.enter_context(tc.tile_pool(name="psum_qk", bufs=3, space="PSUM"))
    psum_sm = ctx.enter_context(tc.tile_pool(name="psum_sm", bufs=2, space="PSUM"))

    # ---- all input DMAs first; order = first-needed first.  The first
    # window's tensors are issued from otherwise-idle engine queues so the
    # issues themselves run in parallel during framework boot.
    wqk_sb = const.tile([128, NCC, 2 * H], BF)
    nc.sync.dma_start(out=wqk_sb, in_=wqk.rearrange("(cc p) h -> p cc h", p=128))
    xTs = []
    for w in range(NW):
        xTw = big.tile([128, NCC, 512], BF, tag=f"xT{w}")
        xTs.append(xTw)

    def dma_xt(w, nsplit, engines=None):
        src = xt[w].rearrange("p (cc t) -> p cc t", cc=NCC)
        step = NCC // nsplit
        for s in range(nsplit):
            eng = nc.sync if engines is None else engines[s]
            eng.dma_start(out=xTs[w][:, s * step:(s + 1) * step, :],
                          in_=src[:, s * step:(s + 1) * step, :])

    dma_xt(0, 6, engines=[nc.scalar, nc.sync, nc.scalar,
                          nc.sync, nc.scalar, nc.sync])
    wv_sb = const.tile([128, NCC, H], BF)
    nc.sync.dma_start(out=wv_sb, in_=wv.rearrange("(cc p) h -> p cc h", p=128))
    dma_xt(1, 6, engines=[nc.scalar, nc.sync, nc.scalar,
                          nc.sync, nc.scalar, nc.sync])
    rt_sb = const.tile([128, RTW], BF)
    # low columns feed windows 0-1 (needed first), high columns windows 2-3
    nc.scalar.dma_start(out=rt_sb[:, 0:768], in_=rt[:, 0:768])
    nc.scalar.dma_start(out=rt_sb[:, 768:1536], in_=rt[:, 768:1536])
    nc.sync.dma_start(out=rt_sb[:, 1536:2048], in_=rt[:, 1536:2048])
    dma_xt(2, 3, engines=[nc.scalar, nc.sync, nc.sync])
    nc.sync.dma_start(out=rt_sb[:, 2048:2560], in_=rt[:, 2048:2560])
    dma_xt(3, 3, engines=[nc.scalar, nc.sync, nc.sync])

    # PE prewarm: dependency-free matmuls during the DMA head keep the HAM
    # clock gate at full rate when real work starts.
    warm = const.tile([128, 512], BF)
    nc.vector.memset(warm, 0.0)
    for _ in range(NWARM):
        pw = psum_qk.tile([128, 512], F32, tag="mm")
        nc.tensor.matmul(pw, lhsT=warm[:, 0:128], rhs=warm, start=True, stop=True,
                         skip_group_check=True)

    qT = big.tile([64, T], BF)            # pre-scaled
    kT = big.tile([64, T], BF)
    v_sb = big.tile([128, NT, H + 1], BF)
    nc.vector.memset(v_sb[:, :, H:H + 1], 1.0)

    pv4s = {}

    def emit_proj(w):
        qsl = slice(w * 512, (w + 1) * 512)
        xTw = xTs[w]
        pm = psum_qk.tile([128, 512], F32, tag="mm")
        for cc in range(NCC):
            nc.tensor.matmul(pm, lhsT=wqk_sb[:, cc, :], rhs=xTw[:, cc, :],
                             start=(cc == 0), stop=(cc == NCC - 1),
                             skip_group_check=True)
        nc.vector.tensor_copy(out=kT[:, qsl], in_=pm[64:128, :])
        nc.scalar.activation(out=qT[:, qsl], in_=pm[0:64, :], func=AF.Copy)
        # direct transposed v-projection: v'[k-chunk, h] = xT-chunk.T @ Wv
        pv4 = psum_sm.tile([128, 4, H], F32, tag="sm")
        for j in range(4):
            for cc in range(NCC):
                nc.tensor.matmul(pv4[:, j, :],
                                 lhsT=xTw[:, cc, j * 128:(j + 1) * 128],
                                 rhs=wv_sb[:, cc, :],
                                 start=(cc == 0), stop=(cc == NCC - 1),
                                 skip_group_check=True)
        pv4s[w] = pv4

    def emit_vevac(w):
        # deferred so it does not sit between the k-evac and the first QK
        # consumer in the DVE queue (cross-engine waits are op-count based)
        nc.vector.tensor_copy(out=v_sb[:, 4 * w:4 * w + 4, 0:H], in_=pv4s[w])

    def emit_qk_pair(w, kp):
        # causal narrowing at pair granularity: columns below off are fully
        # masked for both banks of the pair.
        q0 = w * 512
        kc = 2 * kp
        off = max(0, kc * 128 - q0)
        wd = 512 - off
        ps2 = psum_qk.tile([128, 2, 512], F32, tag="mm")
        for b in range(2):
            k0 = (kc + b) * 128
            nc.tensor.matmul(ps2[:, b, off:512],
                             lhsT=kT[:, k0:k0 + 128],
                             rhs=qT[:, q0 + off:q0 + 512],
                             start=True, stop=True,
                             skip_group_check=True)
        j0 = 512 + q0 - kc * 128
        at2 = work.tile([128, 2, 512], BF, tag="at")
        nc.scalar.activation(out=at2[:, :, off:512], in_=ps2[:, :, off:512],
                             func=AF.Exp)
        # bias * causal mask, multiplicatively, in bf16 (DVE 2x mode)
        nc.vector.tensor_mul(at2[:, :, off:512], at2[:, :, off:512],
                             _rt_pair_ap(rt_sb, j0 + off, wd))
        at_state[w].append(at2)
        if kp == 0:
            emit_vevac(w)

    pv_state = {}

    def emit_pv_pair(w, kp):
        # incremental PV: the two 512-col streams for pair kp accumulate
        # into this window's [65, 512] out psum.  Each consumes only its own
        # pair's bias-multiplied attn tile, so PV chunks pipeline with the
        # exp/mult drain instead of waiting for the whole window.
        q0 = w * 512
        nkc = 4 * (w + 1)
        if kp == 0:
            pv_state[w] = psum_sm.tile([H + 1, 512], F32, tag="sm",
                                       name=f"po{w}")
        po = pv_state[w]
        at2 = at_state[w][kp]
        for b in range(2):
            kc = 2 * kp + b
            bo = max(0, kc * 128 - q0)
            nc.tensor.matmul(po[:, bo:512], lhsT=v_sb[:, kc, :],
                             rhs=at2[:, b, bo:512],
                             start=(kc == 0), stop=(kc == nkc - 1),
                             skip_group_check=True)
        last = nkc // 2 - 1
        if w == NW - 1 and kp == last - 1:
            # final window: columns [0, 256) are complete before the last
            # pair (its chunks only touch [256, 512)), so most of the out
            # evacuation overlaps the last PV pair instead of serializing
            # after it.  stop= is sim-only; the PSUM data is final.
            ob_a = work.tile([H + 1, 256], F32, tag="ob")
            nc.vector.tensor_copy(out=ob_a, in_=po[:, 0:256])
            nc.sync.dma_start(out=out[:, q0:q0 + 256], in_=ob_a)
        elif kp == last:
            if w == NW - 1:
                ob_b = work.tile([H + 1, 256], F32, tag="ob")
                nc.vector.tensor_copy(out=ob_b, in_=po[:, 256:512])
                nc.sync.dma_start(out=out[:, q0 + 256:q0 + 512], in_=ob_b)
            else:
                ob = work.tile([H + 1, 512], F32, tag="ob")
                nc.vector.tensor_copy(out=ob, in_=po)
                nc.sync.dma_start(out=out[:, q0:q0 + 512], in_=ob)

    # software-pipelined loop.  Window w on the PE alternates QK pairs with
    # the PV chunks of two-pairs-ago (whose exp/mult has had time to drain),
    # so the PE stays rate-matched with the Scalar engine's exp stream.
    # proj(w+1) lands mid-window so its psum evacuations drain on DVE/ACT
    # long before window w+1 consumes them.
    at_state = {}
    emit_proj(0)
    for w in range(NW):
        npairs = 2 * (w + 1)
        at_state[w] = []
        emit_qk_pair(w, 0)
        emit_qk_pair(w, 1)
        if w == 0:
            emit_pv_pair(w, 0)
            # keep the HAM clock warm while proj(1) waits on the xt1 DMA
            for _ in range(9):
                pw2 = psum_qk.tile([128, 256], F32, tag="mm", name="pw2")
                nc.tensor.matmul(pw2, lhsT=warm[:, 0:128], rhs=warm[:, 0:256],
                                 start=True, stop=True, skip_group_check=True)
            emit_proj(1)
            emit_pv_pair(w, 1)
            continue
        for kp in range(2, npairs):
            emit_qk_pair(w, kp)
            emit_pv_pair(w, kp - 2)
            if kp == 2 and w + 1 < NW:
                emit_proj(w + 1)
        emit_pv_pair(w, npairs - 2)
        emit_pv_pair(w, npairs - 1)


def make_host_inputs(input_tensor, Wq, Wk, Wv, bias_table):
    x = np.asarray(input_tensor, dtype=np.float32)
    scale = 1.0 / np.sqrt(H)
    wqk = np.concatenate([np.asarray(Wq, dtype=np.float32) * scale,
                          np.asarray(Wk, dtype=np.float32)], axis=1)
    wqk_bf = np.ascontiguousarray(wqk.astype(ml_dtypes.bfloat16))
    wv_bf = np.ascontiguousarray(np.asarray(Wv, dtype=np.float32).astype(ml_dtypes.bfloat16))
    tb = np.asarray(bias_table, dtype=np.float32)[:, 0]
    p = np.arange(128)[:, None]
    j = np.arange(RTW)[None, :]
    idx = j - 512 - p
    rtm = np.where((idx >= 0) & (idx < 2048),
                   np.exp(tb[np.clip(idx, 0, 2047)]),
                   np.float32(0.0)).astype(ml_dtypes.bfloat16)
    rtm = np.ascontiguousarray(rtm)
    # per-core transposed bf16 input, window-major, partition-major rows:
    # [NW, 128, NCC*512] so each window is one contiguous-per-partition DMA
    xts = []
    for i in range(x.shape[0]):
        xt = x[i].T.astype(ml_dtypes.bfloat16)          # [C, T]
        xtw = xt.reshape(NCC, 128, NW, 512)             # [cc, p, w, t]
        xts.append(np.ascontiguousarray(
            xtw.transpose(2, 1, 0, 3).reshape(NW, 128, NCC * 512)))
    return xts, wqk_bf, wv_bf, rtm


def finish_host(raw):
    """raw: [65, T] f32 -> [T, H] f32 (divide by denominator row, transpose)."""
    return np.ascontiguousarray((raw[0:H, :] / raw[H:H + 1, :]).T)


_NC_CACHE = {}


def kernel(input_tensor, Wq, Wk, Wv, bias_table):
    from concourse.bass_utils import run_bass_kernel_spmd
    xts, wqk_bf, wv_bf, rtm = make_host_inputs(input_tensor, Wq, Wk, Wv, bias_table)
    if "nc" not in _NC_CACHE:
        _NC_CACHE["nc"] = build_nc()
    nc = _NC_CACHE["nc"]
    in_maps = [{"xt": xts[i], "wqk": wqk_bf, "wv": wv_bf, "rt": rtm}
               for i in range(NCORES)]
    res = run_bass_kernel_spmd(nc, in_maps, list(range(NCORES)))
    return np.stack([finish_host(np.asarray(res.results[i]["out"], dtype=np.float32))
                     for i in range(NCORES)], axis=0)
